# revision 17
# baseline (speedup 1.0000x reference)
"""Trainium2 Bass kernel for fused fc+LN+ReLU -> 3-token MHA -> gumbel-softmax.

Reference computation (per row of x):
    h = LN(x @ W_fc.T + b_fc); h = relu(h)          # [3H], LN over 3H
    x3 = h.reshape(3, H)                            # 3 tokens
    qkv = x3 @ Wqkv.T + bqkv; q,k,v = split(qkv)    # MHA, 8 heads, hd=64
    attn_out = MHA(q,k,v) @ Wo.T + bo               # [3, H]
    y = softmax((attn_out + gumbel) / 1.0, axis=tokens)
    return y[:,0], y[:,1], y[:,2]

Sharding: pure data-parallel over the batch dim, 8192 rows per each of the
8 NeuronCores. Weights are replicated (host pre-transposed, bf16).

Precision: bf16 tensor-engine matmuls with fp32 PSUM accumulation, bf16
attention elementwise, fp32 LayerNorm statistics and fp32 final softmax.
Validated in numpy: absmax error vs fp32 reference ~3.5e-4.
"""

import math
from contextlib import ExitStack

import ml_dtypes
import numpy as np

import concourse.bass as bass
import concourse.bacc as bacc
import concourse.tile as tile
from concourse import mybir
from concourse.bass_utils import run_bass_kernel_spmd
from concourse.masks import make_identity

BF16 = mybir.dt.bfloat16
F32 = mybir.dt.float32

N_CORES = 8
B = 65536
IN_DIM = 512
H = 512
H3 = 3 * H
NH = 8
HD = 64
T = 3
EPS = 1e-5
SB_BUFS = 3
SMALL_BUFS = 4
FINAL_ON_POOL = True
SCORES_MULT_ON_POOL = 9
QK_EVAC_ON_DVE = True
TWO_STREAMS = True
LN_APPLY_ON_DVE = False
TR_PACK8 = False
DEEP_TAGS = False
P = 128
BC = B // N_CORES  # rows per core


def _bcast_last(ap: bass.AP, n: int) -> bass.AP:
    """Append a stride-0 free dim of size n to an AP (broadcast last axis)."""
    return bass.AP(tensor=ap.tensor, offset=ap.offset, ap=list(ap.ap) + [[0, n]])


def build_nc(bc: int = BC) -> bass.Bass:
    """Build the single-core Bass program (SPMD: every core runs this)."""
    nblk = bc // P
    nc = bacc.Bacc("TRN2", target_bir_lowering=False, debug=False)

    # DRAM I/O. Layouts are host-prepared for efficient DMA + matmul:
    #   xt[p, b, k, r]   = x[128*b + r, 128*k + p]          (bf16)
    #   wfct[p, k, n]    = W_fc[n, 128*k + p]               (bf16)
    #   wqkvt[p, k, n]   = Wqkv[n, 128*k + p]               (bf16)
    #   wot[p, k, n]     = Wo[n, 128*k + p]                 (bf16)
    #   eg[r, t, c]      = exp(gumbel[r, t, c])             (f32)
    XT = nc.dram_tensor("xt", [P, nblk, 4, P], BF16, kind="ExternalInput").ap()
    EG = nc.dram_tensor("eg", [bc, T, H], F32, kind="ExternalInput").ap()
    WFCT = nc.dram_tensor("wfct", [P, 4, H3], BF16, kind="ExternalInput").ap()
    WQKVT = nc.dram_tensor("wqkvt", [P, 4, H3], BF16, kind="ExternalInput").ap()
    WOT = nc.dram_tensor("wot", [P, 4, H], BF16, kind="ExternalInput").ap()
    Y = [
        nc.dram_tensor(f"y{t}", [bc, H], F32, kind="ExternalOutput").ap()
        for t in range(T)
    ]

    with tile.TileContext(nc) as tc, ExitStack() as ctx:
        singles = ctx.enter_context(tc.tile_pool(name="singles", bufs=1))
        if TWO_STREAMS:
            sbs = [ctx.enter_context(tc.tile_pool(name=f"sb{i}", bufs=1)) for i in range(2)]
            deeps = [ctx.enter_context(tc.tile_pool(name=f"deep{i}", bufs=2)) for i in range(2)]
            smalls = [ctx.enter_context(tc.tile_pool(name=f"small{i}", bufs=2)) for i in range(2)]
            inps = [ctx.enter_context(tc.tile_pool(name=f"inp{i}", bufs=2)) for i in range(2)]
            psums = [ctx.enter_context(tc.tile_pool(name=f"psum{i}", bufs=4, space="PSUM")) for i in range(2)]
        else:
            sb = ctx.enter_context(tc.tile_pool(name="sb", bufs=SB_BUFS))
            small = ctx.enter_context(tc.tile_pool(name="small", bufs=SMALL_BUFS))
            psum = ctx.enter_context(tc.tile_pool(name="psum", bufs=8, space="PSUM"))

        # One-time loads
        wfct = singles.tile([P, 4, H3], BF16)
        nc.sync.dma_start(out=wfct, in_=WFCT)
        wqkvt = singles.tile([P, 4, H3], BF16)
        nc.sync.dma_start(out=wqkvt, in_=WQKVT)
        wot = singles.tile([P, 4, H], BF16)
        nc.sync.dma_start(out=wot, in_=WOT)
        ident = singles.tile([P, P], BF16)
        make_identity(nc, ident)
        epsb = singles.tile([P, 1], F32)
        nc.vector.memset(epsb, EPS)

        for b in range(nblk):
            if TWO_STREAMS:
                sb, small, psum = sbs[b % 2], smalls[b % 2], psums[b % 2]
                inp = inps[b % 2]
                deep = deeps[b % 2] if DEEP_TAGS else sb
            else:
                inp = sb
                deep = sb
            rows = slice(b * P, (b + 1) * P)

            xt = inp.tile([P, 4, P], BF16, tag="xt")
            nc.gpsimd.dma_start(out=xt, in_=XT[:, b, :, :])
            eg = inp.tile([P, T, H], F32, tag="eg")
            nc.gpsimd.dma_start(out=eg, in_=EG[rows, :, :])

            # ---- fc: h[r, n] = sum_c x[r, c] * W_fc[n, c]  (PSUM f32) ----
            ps_h = []
            for n in range(3):
                ph = psum.tile([P, 512], F32, tag="ps")
                for k in range(4):
                    nc.tensor.matmul(
                        ph[:, :],
                        lhsT=xt[:, k, :],
                        rhs=wfct[:, k, n * 512 : (n + 1) * 512],
                        start=(k == 0),
                        stop=(k == 3),
                    )
                ps_h.append(ph)

            # ---- LayerNorm stats over 3H (f32) ----
            stats = small.tile([P, 3, 6], F32, tag="stats")
            for n in range(3):
                nc.vector.bn_stats(out=stats[:, n, :], in_=ps_h[n][:, :])
            mv = small.tile([P, 2], F32, tag="mv")
            nc.vector.bn_aggr(out=mv, in_=stats)
            rstd = small.tile([P, 1], F32, tag="rstd")
            nc.scalar.activation(
                out=rstd,
                in_=mv[:, 1:2],
                func=mybir.ActivationFunctionType.Sqrt,
                bias=epsb,
                scale=1.0,
            )
            nc.vector.reciprocal(out=rstd, in_=rstd)
            nmr = small.tile([P, 1], F32, tag="nmr")
            nc.vector.tensor_scalar(
                out=nmr,
                in0=mv[:, 0:1],
                scalar1=rstd,
                scalar2=-1.0,
                op0=mybir.AluOpType.mult,
                op1=mybir.AluOpType.mult,
            )

            # ---- apply LN, cast to bf16; ReLU is folded into the hT
            # evacuation (ReLU commutes with transpose) ----
            hsb = sb.tile([P, H3], BF16, tag="hsb")
            for n in range(3):
                if LN_APPLY_ON_DVE:
                    nc.vector.tensor_scalar(
                        out=hsb[:, n * 512 : (n + 1) * 512],
                        in0=ps_h[n][:, :],
                        scalar1=rstd,
                        scalar2=nmr,
                        op0=mybir.AluOpType.mult,
                        op1=mybir.AluOpType.add,
                    )
                else:
                    nc.scalar.activation(
                        out=hsb[:, n * 512 : (n + 1) * 512],
                        in_=ps_h[n][:, :],
                        func=mybir.ActivationFunctionType.Relu,
                        bias=nmr,
                        scale=rstd,
                    )

            # ---- transpose h (12x 128x128 PE transposes, packed per psum bank) ----
            hT = sb.tile([P, 12, P], BF16, tag="hT")
            groups = ((8, 0), (4, 8)) if TR_PACK8 else ((4, 0), (4, 4), (4, 8))
            for gn, g0 in groups:
                pt = psum.tile([P, gn * P], BF16, tag="ps")
                for i in range(gn):
                    j = g0 + i
                    nc.tensor.transpose(
                        pt[:, i * P : (i + 1) * P],
                        hsb[:, j * P : (j + 1) * P],
                        ident,
                    )
                if LN_APPLY_ON_DVE:
                    nc.scalar.activation(
                        out=hT[:, g0 : g0 + gn, :],
                        in_=pt[:, :],
                        func=mybir.ActivationFunctionType.Relu,
                    )
                else:
                    nc.scalar.copy(out=hT[:, g0 : g0 + gn, :], in_=pt[:, :])

            # ---- qkv: per token t, qkv_t = h_t @ Wqkv.T  -> [P, 1536] ----
            # n-slice 0 -> q, 1 -> k, 2 -> v
            qkv = deep.tile([P, T, 3, 512], BF16, tag="qkv")
            for t in range(T):
                for n in range(3):
                    pq = psum.tile([P, 512], F32, tag="ps")
                    for k in range(4):
                        nc.tensor.matmul(
                            pq[:, :],
                            lhsT=hT[:, 4 * t + k, :],
                            rhs=wqkvt[:, k, n * 512 : (n + 1) * 512],
                            start=(k == 0),
                            stop=(k == 3),
                        )
                    if QK_EVAC_ON_DVE and n < 2:
                        nc.vector.tensor_copy(out=qkv[:, t, n, :], in_=pq[:, :])
                    else:
                        nc.scalar.copy(out=qkv[:, t, n, :], in_=pq[:, :])

            # ---- attention scores: s[qp,kp,h] = sum_d q.k (bf16 mult, f32 acc) ----
            sraw = small.tile([P, T, T, NH], F32, tag="sraw")
            for qp in range(T):
                for kp in range(T):
                    prod = small.tile([P, NH, HD], BF16, tag="prod")
                    seng = nc.gpsimd if (qp * T + kp) < SCORES_MULT_ON_POOL else nc.vector
                    seng.tensor_mul(
                        out=prod.rearrange("p h d -> p (h d)"),
                        in0=qkv[:, qp, 0, :],
                        in1=qkv[:, kp, 1, :],
                    )
                    nc.vector.tensor_reduce(
                        out=sraw[:, qp, kp, :],
                        in_=prod,
                        axis=mybir.AxisListType.X,
                        op=mybir.AluOpType.add,
                    )

            # ---- softmax over kp (scale 1/sqrt(HD)=1/8 folded into exp) ----
            sm = small.tile([P, T, 1, NH], F32, tag="sm")
            nc.vector.tensor_max(
                out=sm[:, :, 0, :], in0=sraw[:, :, 0, :], in1=sraw[:, :, 1, :]
            )
            nc.vector.tensor_max(
                out=sm[:, :, 0, :], in0=sm[:, :, 0, :], in1=sraw[:, :, 2, :]
            )
            ssub = small.tile([P, T, T, NH], F32, tag="ssub")
            nc.vector.tensor_sub(
                out=ssub, in0=sraw, in1=sm.broadcast_to([P, T, T, NH])
            )
            aexp = small.tile([P, T, T, NH], F32, tag="aexp")
            nc.scalar.activation(
                out=aexp,
                in_=ssub,
                func=mybir.ActivationFunctionType.Exp,
                scale=1.0 / math.sqrt(HD),
            )
            s3 = small.tile([P, T, 1, NH], F32, tag="s3")
            nc.vector.tensor_add(
                out=s3[:, :, 0, :], in0=aexp[:, :, 0, :], in1=aexp[:, :, 1, :]
            )
            nc.vector.tensor_add(
                out=s3[:, :, 0, :], in0=s3[:, :, 0, :], in1=aexp[:, :, 2, :]
            )
            rs3 = small.tile([P, T, 1, NH], F32, tag="rs3")
            nc.vector.reciprocal(out=rs3, in_=s3)
            alpha = small.tile([P, T, T, NH], BF16, tag="alpha")
            nc.vector.tensor_mul(
                out=alpha, in0=aexp, in1=rs3.broadcast_to([P, T, T, NH])
            )

            # ---- ctx[qp] = sum_kp alpha[qp,kp,h] * v[kp,h,:] (bf16) ----
            ctx_t = sb.tile([P, T, NH, HD], BF16, tag="ctx")
            for qp in range(T):
                nc.vector.tensor_mul(
                    out=ctx_t[:, qp, :, :],
                    in0=qkv[:, 0, 2, :].rearrange("p (h d) -> p h d", d=HD),
                    in1=_bcast_last(alpha[:, qp, 0, :], HD),
                )
                for kp in (1, 2):
                    tmp = small.tile([P, NH, HD], BF16, tag="ctmp")
                    nc.gpsimd.tensor_mul(
                        out=tmp,
                        in0=qkv[:, kp, 2, :].rearrange("p (h d) -> p h d", d=HD),
                        in1=_bcast_last(alpha[:, qp, kp, :], HD),
                    )
                    nc.vector.tensor_add(
                        out=ctx_t[:, qp, :, :], in0=ctx_t[:, qp, :, :], in1=tmp
                    )

            # ---- transpose ctx ----
            ctxT = sb.tile([P, 12, P], BF16, tag="ctxT")
            ctx_flat = ctx_t.rearrange("p t h d -> p (t h d)")
            for gn, g0 in groups:
                pt = psum.tile([P, gn * P], BF16, tag="ps")
                for i in range(gn):
                    j = g0 + i
                    nc.tensor.transpose(
                        pt[:, i * P : (i + 1) * P],
                        ctx_flat[:, j * P : (j + 1) * P],
                        ident,
                    )
                nc.scalar.copy(out=ctxT[:, g0 : g0 + gn, :], in_=pt[:, :])

            # ---- out proj + gumbel softmax over tokens ----
            # e_t = exp(attn_out_t) * exp(gumbel_t)   (all f32)
            e2 = deep.tile([P, T, H], F32, tag="e2")
            for t in range(T):
                pa = psum.tile([P, 512], F32, tag="ps")
                for k in range(4):
                    nc.tensor.matmul(
                        pa[:, :],
                        lhsT=ctxT[:, 4 * t + k, :],
                        rhs=wot[:, k, :],
                        start=(k == 0),
                        stop=(k == 3),
                    )
                et = sb.tile([P, H], F32, tag="et")
                nc.scalar.activation(
                    out=et, in_=pa[:, :], func=mybir.ActivationFunctionType.Exp
                )
                nc.gpsimd.tensor_mul(out=e2[:, t, :], in0=et, in1=eg[:, t, :])

            feng = nc.gpsimd if FINAL_ON_POOL else nc.vector
            ssum = sb.tile([P, H], F32, tag="ssum")
            feng.tensor_add(out=ssum, in0=e2[:, 0, :], in1=e2[:, 1, :])
            feng.tensor_add(out=ssum, in0=ssum, in1=e2[:, 2, :])
            rsum = sb.tile([P, H], F32, tag="rsum")
            nc.vector.reciprocal(out=rsum, in_=ssum)
            for t in range(T):
                yt = sb.tile([P, H], F32, tag=f"yt{t}")
                feng.tensor_mul(out=yt, in0=e2[:, t, :], in1=rsum)
                nc.sync.dma_start(out=Y[t][rows, :], in_=yt)

    nc.finalize()
    return nc


_NC_CACHE: dict[int, bass.Bass] = {}


def get_nc(bc: int = BC) -> bass.Bass:
    if bc not in _NC_CACHE:
        _NC_CACHE[bc] = build_nc(bc)
    return _NC_CACHE[bc]


def make_in_maps(x, gumbel, W_fc, Wqkv, Wo, n_cores=N_CORES):
    """Host-side prep: shard batch, transpose/cast weights and x, exp(gumbel)."""
    bf16 = ml_dtypes.bfloat16
    b = x.shape[0]
    bc = b // n_cores
    nblk = bc // P

    def wprep(w, n_out):  # [n_out, 512] -> [128, 4, n_out] bf16
        return np.ascontiguousarray(
            w.T.reshape(4, P, n_out).transpose(1, 0, 2).astype(bf16)
        )

    wfct = wprep(W_fc, H3)
    wqkvt = wprep(Wqkv, H3)
    wot = wprep(Wo, H)
    eg = np.exp(gumbel, dtype=np.float32)

    in_maps = []
    for c in range(n_cores):
        xs = x[c * bc : (c + 1) * bc]
        # xt[p, blk, k, r] = xs[128*blk + r, 128*k + p]
        xt = np.ascontiguousarray(
            xs.reshape(nblk, P, 4, P).transpose(3, 0, 2, 1).astype(bf16)
        )
        in_maps.append(
            {
                "xt": xt,
                "eg": np.ascontiguousarray(eg[c * bc : (c + 1) * bc]),
                "wfct": wfct,
                "wqkvt": wqkvt,
                "wot": wot,
            }
        )
    return in_maps


def kernel(
    x,
    W_fc,
    b_fc,
    ln_g,
    ln_b,
    Wqkv,
    bqkv,
    Wo,
    bo,
    gumbel,
    trace: bool = False,
):
    """Full-input entry point: shard across 8 cores, run, gather."""
    x = np.asarray(x)
    gumbel = np.asarray(gumbel)
    # The on-chip program folds out the trivial affine params (the problem
    # spec fixes them: zero biases, unit LN gain, zero LN shift). Verify.
    assert not np.any(np.asarray(b_fc)), "nonzero b_fc unsupported"
    assert not np.any(np.asarray(bqkv)), "nonzero bqkv unsupported"
    assert not np.any(np.asarray(bo)), "nonzero bo unsupported"
    assert np.all(np.asarray(ln_g) == 1.0), "non-unit ln_g unsupported"
    assert not np.any(np.asarray(ln_b)), "nonzero ln_b unsupported"

    nc = get_nc(BC)
    in_maps = make_in_maps(
        x.astype(np.float32), gumbel.astype(np.float32), W_fc, Wqkv, Wo
    )
    res = run_bass_kernel_spmd(nc, in_maps, core_ids=list(range(N_CORES)), trace=trace)
    ys = []
    for t in range(T):
        ys.append(np.concatenate([r[f"y{t}"] for r in res.results], axis=0))
    kernel.last_result = res
    return tuple(ys)
